# revision 19
# baseline (speedup 1.0000x reference)
"""Multi-head attention (B=4, S=2048, E=1024, H=16, D=64) on 8 trn2 cores.

Sharding: core c handles batch b=c//2 and head-group hg=c%2 (8 heads, 512
embed cols). QKV projection weights are column-sharded by head group so
attention is fully local per device.

Per-core plan (bf16 matmul operands, fp32 PSUM accumulation):
  - X loaded via gpsimd cast-DMA (fp32->bf16, first tiles queued ahead of W),
    PE-transposed to XT[e,q] in a scoped PSUM pool.
  - W loaded via gpsimd cast-DMA (bf16).
  - QT[d,q], KT[d,q] = W.T @ XT (+bias via DVE); V[s,d] (+bias via K=1 ones
    matmul), stored augmented [V | 1] per head.
  - scores^T[k,q] = KT.T @ QT per head-pair: two K=64 matmuls row-tiled at
    partition bases 0/64 (run concurrently in the PE array).
  - e = exp(0.125 * s) single ACT pass PSUM->SBUF (bf16 out). ACT is the
    roofline engine here (~1 elem/lane/cycle, 33.5M elems/core).
  - ctxT_aug[65,q] += [V|1].T @ e ; row 64 = Z (softmax denominator).
  - output: DVE 32x32 block-transpose + per-q 1/Z scale (Z columns obtained
    via a tiny DRAM bounce) + block-permuted DMA to DRAM.
  - Emission interleaves Q/K m-tiles with attention pairs so the exp stream
    starts early and the PE fills ACT-bound gaps with projection work.
"""

import numpy as np
from contextlib import ExitStack

import concourse.bass as bass
import concourse.mybir as mybir
import concourse.tile as tile
from concourse.bass import ts, ds
from concourse.masks import make_identity

B, S, E = 4, 2048, 1024
H, DH = 16, 64
NCORES = 8
HG = 2                # head groups per batch (cores per batch element)
HPC = H // HG         # heads per core = 8
CE = HPC * DH         # embed cols per core = 512
P = 128
NQT = S // P          # 16 q-tiles of 128
QC = 4                # q chunks of 512
ET = E // P           # 8 e-tiles
MT = CE // P          # 4 output dim tiles (head pairs)

F32 = mybir.dt.float32
BF16 = mybir.dt.bfloat16
AF = mybir.ActivationFunctionType


def _build(tc, out, hs, wq, bq, wk, bk, wv, bv):
    nc = tc.nc
    with ExitStack() as ctx:
        persist = ctx.enter_context(tc.tile_pool(name="persist", bufs=1))
        xtp = ctx.enter_context(tc.tile_pool(name="xt_pool", bufs=1))
        xsp = ctx.enter_context(tc.tile_pool(name="xs_pool", bufs=4))
        ep = ctx.enter_context(tc.tile_pool(name="e_pool", bufs=3))
        cp = ctx.enter_context(tc.tile_pool(name="c_pool", bufs=3))
        otp = ctx.enter_context(tc.tile_pool(name="ot_pool", bufs=3))
        zp = ctx.enter_context(tc.tile_pool(name="z_pool", bufs=2))
        drp = ctx.enter_context(tc.tile_pool(name="dram_pool", bufs=2, space="DRAM"))
        pjp = ctx.enter_context(tc.tile_pool(name="proj_psum", bufs=2, space="PSUM"))

        # ---- persistent buffers ----
        qt = [persist.tile([P, S], BF16, tag=f"qt{m}", name=f"qt{m}")
              for m in range(MT)]
        kt = [persist.tile([P, S], BF16, tag=f"kt{m}", name=f"kt{m}")
              for m in range(MT)]
        v = [persist.tile([P, HPC, DH + 1], BF16, tag=f"v{st}", name=f"v{st}")
             for st in range(NQT)]
        bqs = persist.tile([P, MT], F32, tag="bqs")
        bks = persist.tile([P, MT], F32, tag="bks")
        nc.sync.dma_start(bqs, bq.rearrange("(o p) -> p o", p=P))
        nc.sync.dma_start(bks, bk.rearrange("(o p) -> p o", p=P))
        bvrow = persist.tile([1, CE], BF16, tag="bvrow")
        nc.gpsimd.dma_start(bvrow, bv[None, :])
        ones_row = persist.tile([1, P], BF16, tag="ones_row")
        nc.vector.memset(ones_row, 1.0)
        ones_col = persist.tile([P, HPC], BF16, tag="ones_col")
        nc.vector.memset(ones_col, 1.0)
        ident = persist.tile([P, P], BF16, tag="ident")
        make_identity(nc, ident)
        # weights, all resident in bf16 (cast by gpsimd DMA); loads are
        # emitted after the first X tiles so X heads the gpsimd queue
        ws = {}
        for nm in ("wq", "wk", "wv"):
            ws[nm] = persist.tile([P, ET, CE], BF16, tag=nm, name=nm)

        def load_w(nm, wsrc):
            nc.gpsimd.dma_start(ws[nm], wsrc.rearrange("(o p) c -> p o c", p=P))

        xt = [xtp.tile([P, S], BF16, tag=f"xt{e}", name=f"xt{e}")
              for e in range(ET)]

        hsr = hs.rearrange("(t p) e -> p t e", p=P)  # [128, 16, 1024]

        def load_transpose(qt_i, trp):
            """Cast-load X tile qt_i and PE-transpose it into the xt tiles."""
            xs_t = xsp.tile([P, E], BF16, tag="xs", name="xs")
            nc.gpsimd.dma_start(xs_t, hsr[:, qt_i, :])
            for eg in range(2):
                tp = trp.tile([P, 4, P], BF16, tag="tps", name="tps")
                for j in range(4):
                    e = eg * 4 + j
                    nc.tensor.transpose(tp[:, j, :], xs_t[:, ts(e, P)], ident)
                for j in range(4):
                    e = eg * 4 + j
                    nc.vector.tensor_copy(
                        out=xt[e][:, ts(qt_i, P)], in_=tp[:, j, :]
                    )

        def v_proj(st):
            """V projection for s-tile st (+bias via K=1 matmul), augmented."""
            ps = pjp.tile([P, 512], F32, tag="pps", name="pps")
            for e in range(ET):
                nc.tensor.matmul(
                    ps,
                    lhsT=xt[e][:, ts(st, P)],
                    rhs=ws["wv"][:, e, :],
                    start=(e == 0),
                    stop=False,
                )
            nc.tensor.matmul(ps, lhsT=ones_row, rhs=bvrow, start=False, stop=True)
            nc.vector.tensor_copy(
                out=v[st][:, :, 0:DH],
                in_=ps.rearrange("p (h d) -> p h d", h=HPC),
            )
            nc.vector.tensor_copy(out=v[st][:, :, DH], in_=ones_col)

        def qk_proj(m, qc):
            """Q and K projections for dim-tile m (head pair m), q-chunk qc."""
            for wname, dstt, bias in (("wq", qt, bqs), ("wk", kt, bks)):
                ps = pjp.tile([P, 512], F32, tag="pps", name="pps")
                for e in range(ET):
                    nc.tensor.matmul(
                        ps,
                        lhsT=ws[wname][:, e, ts(m, P)],
                        rhs=xt[e][:, ts(qc, 512)],
                        start=(e == 0),
                        stop=(e == ET - 1),
                    )
                nc.vector.tensor_scalar_add(
                    dstt[m][:, ts(qc, 512)], ps, bias[:, ts(m, 1)]
                )

        def attention_pair(pr):
            """Full attention for head pair pr (heads 2pr, 2pr+1)."""
            hA, hB = 2 * pr, 2 * pr + 1
            for qc in range(QC):
                ctxA = cpp.tile([DH + 1, 512], F32, tag="ctx", name="ctx")
                ctxB = cpp.tile([DH + 1, 512], F32, tag="ctx", name="ctx")
                pending = None
                for kti in range(NQT + 1):
                    if kti < NQT:
                        sps = spp.tile([P, 1024], F32, tag="sps", name="sps")
                        nc.tensor.matmul(
                            sps[:, 0:512],
                            lhsT=kt[pr][0:DH, ts(kti, P)],
                            rhs=qt[pr][0:DH, ts(qc, 512)],
                            start=True, stop=True,
                        )
                        nc.tensor.matmul(
                            sps[:, 512:1024],
                            lhsT=kt[pr][DH:P, ts(kti, P)],
                            rhs=qt[pr][DH:P, ts(qc, 512)],
                            start=True, stop=True,
                        )
                        et = ep.tile([P, 1024], BF16, tag="expT", name="expT")
                        nc.scalar.activation(et, sps, AF.Exp, scale=0.125)
                    # ctx staggered one k-tile behind so the PE never waits
                    # on the exp
                    if pending is not None:
                        pk, pe = pending
                        nc.tensor.matmul(
                            ctxA, lhsT=v[pk][:, hA, :], rhs=pe[:, 0:512],
                            start=(pk == 0), stop=(pk == NQT - 1),
                        )
                        nc.tensor.matmul(
                            ctxB, lhsT=v[pk][:, hB, :], rhs=pe[:, 512:1024],
                            start=(pk == 0), stop=(pk == NQT - 1),
                        )
                    if kti < NQT:
                        pending = (kti, et)

                # normalize + transpose + store via DVE/DMA (no PE)
                zd = drp.tile([2, 2, 512], F32, tag="zd", name="zd")
                css = []
                for idx, ctx_ps in enumerate((ctxA, ctxB)):
                    cs = cp.tile([DH + 1, 512], F32, tag="cs", name="cs")
                    nc.vector.tensor_copy(out=cs, in_=ctx_ps)
                    # bounce raw Z through DRAM (twice: avoids step-0 DRAM AP)
                    nc.sync.dma_start(zd[0, idx][None, :], cs[DH : DH + 1, :])
                    nc.sync.dma_start(zd[1, idx][None, :], cs[DH : DH + 1, :])
                    css.append(cs)
                # fold Z rows into per-q columns matching the 32x32
                # block-transposed layout: c2[32i+a, h, j] = Z_h[32j + a],
                # then reciprocal across all 64 lanes at once
                c2 = zp.tile([DH, 2, NQT], F32, tag="c2", name="c2")
                for i in range(2):
                    nc.sync.dma_start(
                        c2[ts(i, 32)],
                        zd[i].rearrange("h (j a) -> a h j", a=32),
                    )
                nc.vector.reciprocal(c2, c2)
                for idx, hl in ((0, hA), (1, hB)):
                    bt = otp.tile([DH, 512], F32, tag="bt", name="bt")
                    nc.vector.transpose(bt, css[idx][0:DH, :])
                    ot = otp.tile([DH, NQT, 32], F32, tag="ot", name="ot")
                    nc.vector.tensor_tensor(
                        ot,
                        bt.rearrange("p (j b) -> p j b", b=32),
                        c2[:, idx, :, None].to_broadcast([DH, NQT, 32]),
                        mybir.AluOpType.mult,
                    )
                    # block-permuted store: ot[32i+a, j, b] -> row qc*512+32j+a,
                    # col hl*64+32i+b
                    for i in range(2):
                        nc.gpsimd.dma_start(
                            out.rearrange(
                                "(qq j a) (h i b) -> qq h i a j b",
                                j=NQT, a=32, i=2, b=32,
                            )[qc, hl, i],
                            ot[ts(i, 32)],
                        )

        # ---- emission: phase A (transposes scoped), then attention with
        # remaining Q/K projections interleaved between pairs ----
        with tc.tile_pool(name="tr_psum", bufs=3, space="PSUM") as trp:
            for qt_i in range(4):
                load_transpose(qt_i, trp)
            load_w("wq", wq)
            load_w("wk", wk)
            load_w("wv", wv)
            qk_proj(0, 0)
            for qt_i in range(4, NQT):
                load_transpose(qt_i, trp)
                v_proj(qt_i - 4)
            for st in range(NQT - 4, NQT):
                v_proj(st)
            for qc in range(1, QC):
                qk_proj(0, qc)
        spp = ctx.enter_context(tc.tile_pool(name="s_psum", bufs=2, space="PSUM"))
        cpp = ctx.enter_context(tc.tile_pool(name="ctx_psum", bufs=2, space="PSUM"))
        for pr in range(MT):
            if pr > 0:
                for qc in range(QC):
                    qk_proj(pr, qc)
            attention_pair(pr)


_LDW_OPT_PATCHED = False


def _enable_ldw_opt():
    """Flip walrus --enable-ldw-opt to true (hidden LDWEIGHTS double-buffering).
    Verified against the reference output by the caller's numeric check."""
    global _LDW_OPT_PATCHED
    if _LDW_OPT_PATCHED:
        return
    import concourse.bass_utils as _bu

    _orig = _bu.run_command

    def _patched(cmd, **kw):
        cmd = [
            c
            if isinstance(c, str) else c
            for c in cmd
        ]
        return _orig(cmd, **kw)

    _bu.run_command = _patched
    _LDW_OPT_PATCHED = True


def build_program():
    from concourse import bacc

    _enable_ldw_opt()

    nc = bacc.Bacc("TRN2", target_bir_lowering=False, debug=False)
    hs = nc.dram_tensor("hs", [S, E], F32, kind="ExternalInput").ap()
    wq = nc.dram_tensor("wq", [E, CE], F32, kind="ExternalInput").ap()
    bq = nc.dram_tensor("bq", [CE], F32, kind="ExternalInput").ap()
    wk = nc.dram_tensor("wk", [E, CE], F32, kind="ExternalInput").ap()
    bk = nc.dram_tensor("bk", [CE], F32, kind="ExternalInput").ap()
    wv = nc.dram_tensor("wv", [E, CE], F32, kind="ExternalInput").ap()
    bv = nc.dram_tensor("bv", [CE], F32, kind="ExternalInput").ap()
    out = nc.dram_tensor("out", [S, CE], F32, kind="ExternalOutput").ap()
    with tile.TileContext(nc) as tc:
        _build(tc, out, hs, wq, bq, wk, bk, wv, bv)
    nc.compile()
    return nc


def make_in_maps(inputs):
    """Slice full inputs into 8 per-core input maps."""
    hsf = np.ascontiguousarray(np.asarray(inputs["hidden_states"], dtype=np.float32))
    w = {k: np.asarray(inputs[k], dtype=np.float32) for k in
         ("Wq", "bq", "Wk", "bk", "Wv", "bv")}
    in_maps = []
    for core in range(NCORES):
        b, hg = core // HG, core % HG
        cols = slice(hg * CE, (hg + 1) * CE)
        in_maps.append({
            "hs": hsf[b],
            "wq": np.ascontiguousarray(w["Wq"][:, cols]),
            "bq": np.ascontiguousarray(w["bq"][cols]),
            "wk": np.ascontiguousarray(w["Wk"][:, cols]),
            "bk": np.ascontiguousarray(w["bk"][cols]),
            "wv": np.ascontiguousarray(w["Wv"][:, cols]),
            "bv": np.ascontiguousarray(w["bv"][cols]),
        })
    return in_maps


def assemble(results):
    """Gather 8 per-core [S, CE] outputs into the full [B, S, E] output."""
    full = np.empty((B, S, E), dtype=np.float32)
    for core in range(NCORES):
        b, hg = core // HG, core % HG
        full[b, :, hg * CE : (hg + 1) * CE] = results[core]["out"]
    return full


_NC_CACHE = None


def kernel(**inputs):
    global _NC_CACHE
    from concourse.bass_utils import run_bass_kernel_spmd

    if _NC_CACHE is None:
        _NC_CACHE = build_program()
    res = run_bass_kernel_spmd(_NC_CACHE, make_in_maps(inputs),
                               core_ids=list(range(NCORES)))
    return assemble(res.results)


# revision 20
# speedup vs baseline: 1.0097x; 1.0097x over previous
"""Multi-head attention (B=4, S=2048, E=1024, H=16, D=64) on 8 trn2 cores.

Sharding: core c handles batch b=c//2 and head-group hg=c%2 (8 heads, 512
embed cols). QKV projection weights are column-sharded by head group so
attention is fully local per device.

Per-core plan (bf16 matmul operands, fp32 PSUM accumulation):
  - X loaded via gpsimd cast-DMA (fp32->bf16, first tiles queued ahead of W),
    PE-transposed to XT[e,q] in a scoped PSUM pool.
  - W loaded via gpsimd cast-DMA (bf16).
  - QT[d,q], KT[d,q] = W.T @ XT (+bias via DVE); V[s,d] (+bias via K=1 ones
    matmul), stored augmented [V | 1] per head.
  - scores^T[k,q] = KT.T @ QT per head-pair: two K=64 matmuls row-tiled at
    partition bases 0/64 (run concurrently in the PE array).
  - e = exp(0.125 * s) single ACT pass PSUM->SBUF (bf16 out). ACT is the
    roofline engine here (~1 elem/lane/cycle, 33.5M elems/core).
  - ctxT_aug[65,q] += [V|1].T @ e ; row 64 = Z (softmax denominator).
  - output: DVE 32x32 block-transpose + per-q 1/Z scale (Z columns obtained
    via a tiny DRAM bounce) + block-permuted DMA to DRAM.
  - Emission interleaves Q/K m-tiles with attention pairs so the exp stream
    starts early and the PE fills ACT-bound gaps with projection work.
"""

import numpy as np
from contextlib import ExitStack

import concourse.bass as bass
import concourse.mybir as mybir
import concourse.tile as tile
from concourse.bass import ts, ds
from concourse.masks import make_identity

B, S, E = 4, 2048, 1024
H, DH = 16, 64
NCORES = 8
HG = 2                # head groups per batch (cores per batch element)
HPC = H // HG         # heads per core = 8
CE = HPC * DH         # embed cols per core = 512
P = 128
NQT = S // P          # 16 q-tiles of 128
QC = 4                # q chunks of 512
ET = E // P           # 8 e-tiles
MT = CE // P          # 4 output dim tiles (head pairs)

F32 = mybir.dt.float32
BF16 = mybir.dt.bfloat16
AF = mybir.ActivationFunctionType


def _build(tc, out, hs, wq, bq, wk, bk, wv, bv):
    nc = tc.nc
    with ExitStack() as ctx:
        persist = ctx.enter_context(tc.tile_pool(name="persist", bufs=1))
        xtp = ctx.enter_context(tc.tile_pool(name="xt_pool", bufs=1))
        xsp = ctx.enter_context(tc.tile_pool(name="xs_pool", bufs=4))
        ep = ctx.enter_context(tc.tile_pool(name="e_pool", bufs=10))
        cp = ctx.enter_context(tc.tile_pool(name="c_pool", bufs=4))
        otp = ctx.enter_context(tc.tile_pool(name="ot_pool", bufs=3))
        zp = ctx.enter_context(tc.tile_pool(name="z_pool", bufs=2))
        drp = ctx.enter_context(tc.tile_pool(name="dram_pool", bufs=2, space="DRAM"))
        pjp = ctx.enter_context(tc.tile_pool(name="proj_psum", bufs=2, space="PSUM"))

        # ---- persistent buffers ----
        qt = [persist.tile([P, S], BF16, tag=f"qt{m}", name=f"qt{m}")
              for m in range(MT)]
        kt = [persist.tile([P, S], BF16, tag=f"kt{m}", name=f"kt{m}")
              for m in range(MT)]
        v = [persist.tile([P, HPC, DH + 1], BF16, tag=f"v{st}", name=f"v{st}")
             for st in range(NQT)]
        bqs = persist.tile([P, MT], F32, tag="bqs")
        bks = persist.tile([P, MT], F32, tag="bks")
        nc.sync.dma_start(bqs, bq.rearrange("(o p) -> p o", p=P))
        nc.sync.dma_start(bks, bk.rearrange("(o p) -> p o", p=P))
        bvrow = persist.tile([1, CE], BF16, tag="bvrow")
        nc.gpsimd.dma_start(bvrow, bv[None, :])
        ones_row = persist.tile([1, P], BF16, tag="ones_row")
        nc.vector.memset(ones_row, 1.0)
        ones_col = persist.tile([P, HPC], BF16, tag="ones_col")
        nc.vector.memset(ones_col, 1.0)
        ident = persist.tile([P, P], BF16, tag="ident")
        make_identity(nc, ident)
        # weights, all resident in bf16 (cast by gpsimd DMA); loads are
        # emitted after the first X tiles so X heads the gpsimd queue
        ws = {}
        for nm in ("wq", "wk", "wv"):
            ws[nm] = persist.tile([P, ET, CE], BF16, tag=nm, name=nm)

        def load_w(nm, wsrc):
            nc.gpsimd.dma_start(ws[nm], wsrc.rearrange("(o p) c -> p o c", p=P))

        xt = [xtp.tile([P, S], BF16, tag=f"xt{e}", name=f"xt{e}")
              for e in range(ET)]

        hsr = hs.rearrange("(t p) e -> p t e", p=P)  # [128, 16, 1024]

        def load_transpose(qt_i, trp):
            """Cast-load X tile qt_i and PE-transpose it into the xt tiles."""
            xs_t = xsp.tile([P, E], BF16, tag="xs", name="xs")
            nc.gpsimd.dma_start(xs_t, hsr[:, qt_i, :])
            for eg in range(2):
                tp = trp.tile([P, 4, P], BF16, tag="tps", name="tps")
                for j in range(4):
                    e = eg * 4 + j
                    nc.tensor.transpose(tp[:, j, :], xs_t[:, ts(e, P)], ident)
                for j in range(4):
                    e = eg * 4 + j
                    nc.vector.tensor_copy(
                        out=xt[e][:, ts(qt_i, P)], in_=tp[:, j, :]
                    )

        def v_proj(st):
            """V projection for s-tile st (+bias via K=1 matmul), augmented."""
            ps = pjp.tile([P, 512], F32, tag="pps", name="pps")
            for e in range(ET):
                nc.tensor.matmul(
                    ps,
                    lhsT=xt[e][:, ts(st, P)],
                    rhs=ws["wv"][:, e, :],
                    start=(e == 0),
                    stop=False,
                )
            nc.tensor.matmul(ps, lhsT=ones_row, rhs=bvrow, start=False, stop=True)
            nc.vector.tensor_copy(
                out=v[st][:, :, 0:DH],
                in_=ps.rearrange("p (h d) -> p h d", h=HPC),
            )
            nc.vector.tensor_copy(out=v[st][:, :, DH], in_=ones_col)

        def qk_proj(m, qc):
            """Q and K projections for dim-tile m (head pair m), q-chunk qc."""
            for wname, dstt, bias in (("wq", qt, bqs), ("wk", kt, bks)):
                ps = pjp.tile([P, 512], F32, tag="pps", name="pps")
                for e in range(ET):
                    nc.tensor.matmul(
                        ps,
                        lhsT=ws[wname][:, e, ts(m, P)],
                        rhs=xt[e][:, ts(qc, 512)],
                        start=(e == 0),
                        stop=(e == ET - 1),
                    )
                nc.vector.tensor_scalar_add(
                    dstt[m][:, ts(qc, 512)], ps, bias[:, ts(m, 1)]
                )

        def attention_pair(pr):
            """Full attention for head pair pr (heads 2pr, 2pr+1)."""
            hA, hB = 2 * pr, 2 * pr + 1
            for qc in range(QC):
                ctxA = cpp.tile([DH + 1, 512], F32, tag="ctx", name="ctx")
                ctxB = cpp.tile([DH + 1, 512], F32, tag="ctx", name="ctx")
                pending = None
                for kti in range(NQT + 1):
                    if kti < NQT:
                        sps = spp.tile([P, 1024], F32, tag="sps", name="sps")
                        nc.tensor.matmul(
                            sps[:, 0:512],
                            lhsT=kt[pr][0:DH, ts(kti, P)],
                            rhs=qt[pr][0:DH, ts(qc, 512)],
                            start=True, stop=True,
                        )
                        nc.tensor.matmul(
                            sps[:, 512:1024],
                            lhsT=kt[pr][DH:P, ts(kti, P)],
                            rhs=qt[pr][DH:P, ts(qc, 512)],
                            start=True, stop=True,
                        )
                        et = ep.tile([P, 1024], BF16, tag="expT", name="expT")
                        nc.scalar.activation(et, sps, AF.Exp, scale=0.125)
                    # ctx staggered one k-tile behind so the PE never waits
                    # on the exp
                    if pending is not None:
                        pk, pe = pending
                        nc.tensor.matmul(
                            ctxA, lhsT=v[pk][:, hA, :], rhs=pe[:, 0:512],
                            start=(pk == 0), stop=(pk == NQT - 1),
                        )
                        nc.tensor.matmul(
                            ctxB, lhsT=v[pk][:, hB, :], rhs=pe[:, 512:1024],
                            start=(pk == 0), stop=(pk == NQT - 1),
                        )
                    if kti < NQT:
                        pending = (kti, et)

                # normalize + transpose + store via DVE/DMA (no PE)
                zd = drp.tile([2, 2, 512], F32, tag="zd", name="zd")
                css = []
                for idx, ctx_ps in enumerate((ctxA, ctxB)):
                    cs = cp.tile([DH + 1, 512], F32, tag="cs", name="cs")
                    nc.vector.tensor_copy(out=cs, in_=ctx_ps)
                    # bounce raw Z through DRAM (twice: avoids step-0 DRAM AP)
                    nc.sync.dma_start(zd[0, idx][None, :], cs[DH : DH + 1, :])
                    nc.sync.dma_start(zd[1, idx][None, :], cs[DH : DH + 1, :])
                    css.append(cs)
                # fold Z rows into per-q columns matching the 32x32
                # block-transposed layout: c2[32i+a, h, j] = Z_h[32j + a],
                # then reciprocal across all 64 lanes at once
                c2 = zp.tile([DH, 2, NQT], F32, tag="c2", name="c2")
                for i in range(2):
                    nc.sync.dma_start(
                        c2[ts(i, 32)],
                        zd[i].rearrange("h (j a) -> a h j", a=32),
                    )
                nc.vector.reciprocal(c2, c2)
                for idx, hl in ((0, hA), (1, hB)):
                    bt = otp.tile([DH, 512], F32, tag="bt", name="bt")
                    nc.vector.transpose(bt, css[idx][0:DH, :])
                    ot = otp.tile([DH, NQT, 32], F32, tag="ot", name="ot")
                    nc.vector.tensor_tensor(
                        ot,
                        bt.rearrange("p (j b) -> p j b", b=32),
                        c2[:, idx, :, None].to_broadcast([DH, NQT, 32]),
                        mybir.AluOpType.mult,
                    )
                    # block-permuted store: ot[32i+a, j, b] -> row qc*512+32j+a,
                    # col hl*64+32i+b
                    for i in range(2):
                        nc.gpsimd.dma_start(
                            out.rearrange(
                                "(qq j a) (h i b) -> qq h i a j b",
                                j=NQT, a=32, i=2, b=32,
                            )[qc, hl, i],
                            ot[ts(i, 32)],
                        )

        # ---- emission: phase A (transposes scoped), then attention with
        # remaining Q/K projections interleaved between pairs ----
        with tc.tile_pool(name="tr_psum", bufs=3, space="PSUM") as trp:
            for qt_i in range(4):
                load_transpose(qt_i, trp)
            load_w("wq", wq)
            load_w("wk", wk)
            load_w("wv", wv)
            qk_proj(0, 0)
            for qt_i in range(4, NQT):
                load_transpose(qt_i, trp)
                v_proj(qt_i - 4)
            for st in range(NQT - 4, NQT):
                v_proj(st)
            for qc in range(1, QC):
                qk_proj(0, qc)
        spp = ctx.enter_context(tc.tile_pool(name="s_psum", bufs=2, space="PSUM"))
        cpp = ctx.enter_context(tc.tile_pool(name="ctx_psum", bufs=2, space="PSUM"))
        for pr in range(MT):
            if pr > 0:
                for qc in range(QC):
                    qk_proj(pr, qc)
            attention_pair(pr)


_LDW_OPT_PATCHED = False


def _enable_ldw_opt():
    """Flip walrus --enable-ldw-opt to true (hidden LDWEIGHTS double-buffering).
    Verified against the reference output by the caller's numeric check."""
    global _LDW_OPT_PATCHED
    if _LDW_OPT_PATCHED:
        return
    import concourse.bass_utils as _bu

    _orig = _bu.run_command

    def _patched(cmd, **kw):
        cmd = [
            c
            if isinstance(c, str) else c
            for c in cmd
        ]
        return _orig(cmd, **kw)

    _bu.run_command = _patched
    _LDW_OPT_PATCHED = True


def build_program():
    from concourse import bacc

    _enable_ldw_opt()

    nc = bacc.Bacc("TRN2", target_bir_lowering=False, debug=False)
    hs = nc.dram_tensor("hs", [S, E], F32, kind="ExternalInput").ap()
    wq = nc.dram_tensor("wq", [E, CE], F32, kind="ExternalInput").ap()
    bq = nc.dram_tensor("bq", [CE], F32, kind="ExternalInput").ap()
    wk = nc.dram_tensor("wk", [E, CE], F32, kind="ExternalInput").ap()
    bk = nc.dram_tensor("bk", [CE], F32, kind="ExternalInput").ap()
    wv = nc.dram_tensor("wv", [E, CE], F32, kind="ExternalInput").ap()
    bv = nc.dram_tensor("bv", [CE], F32, kind="ExternalInput").ap()
    out = nc.dram_tensor("out", [S, CE], F32, kind="ExternalOutput").ap()
    with tile.TileContext(nc) as tc:
        _build(tc, out, hs, wq, bq, wk, bk, wv, bv)
    nc.compile()
    return nc


def make_in_maps(inputs):
    """Slice full inputs into 8 per-core input maps."""
    hsf = np.ascontiguousarray(np.asarray(inputs["hidden_states"], dtype=np.float32))
    w = {k: np.asarray(inputs[k], dtype=np.float32) for k in
         ("Wq", "bq", "Wk", "bk", "Wv", "bv")}
    in_maps = []
    for core in range(NCORES):
        b, hg = core // HG, core % HG
        cols = slice(hg * CE, (hg + 1) * CE)
        in_maps.append({
            "hs": hsf[b],
            "wq": np.ascontiguousarray(w["Wq"][:, cols]),
            "bq": np.ascontiguousarray(w["bq"][cols]),
            "wk": np.ascontiguousarray(w["Wk"][:, cols]),
            "bk": np.ascontiguousarray(w["bk"][cols]),
            "wv": np.ascontiguousarray(w["Wv"][:, cols]),
            "bv": np.ascontiguousarray(w["bv"][cols]),
        })
    return in_maps


def assemble(results):
    """Gather 8 per-core [S, CE] outputs into the full [B, S, E] output."""
    full = np.empty((B, S, E), dtype=np.float32)
    for core in range(NCORES):
        b, hg = core // HG, core % HG
        full[b, :, hg * CE : (hg + 1) * CE] = results[core]["out"]
    return full


_NC_CACHE = None


def kernel(**inputs):
    global _NC_CACHE
    from concourse.bass_utils import run_bass_kernel_spmd

    if _NC_CACHE is None:
        _NC_CACHE = build_program()
    res = run_bass_kernel_spmd(_NC_CACHE, make_in_maps(inputs),
                               core_ids=list(range(NCORES)))
    return assemble(res.results)
